# revision 42
# baseline (speedup 1.0000x reference)
"""FAVOR+ linear attention (Performer-style) Trainium2 Bass kernel, v2.

Full inputs -> full output. Sharding: 8 cores = (batch b in 0..3) x (pixel
half in 0..1), sequence-parallel: each core computes the KV summary over its
own 8192 key pixels, AllReduces KV within the (b, half) pair, and computes
num/den for its 8192 query pixels.

Device-side structure (per core):
  K phase: y [C, 8192] bf16 -> t = scale*Wk y + scale*bk (PE + Act/DVE),
           kf = exp(t'bmat + t2'fmat) (PE + Act), KV += kf' [v|1] (PE).
           v is host-packed pixel-major with a fused ones column.
  AllReduce KV [128,129] across the pair (gpsimd collective via DRAM).
  Q phase: qs = exp(Gl' x + gq) with Gl = (scale*Wq)' bmat — the q-side
           -0.5|t|^2 term cancels in num/den, so the projection and feature
           matmuls fuse into one and no norm is needed.
           num|den = qs' kvb (PE), out = num * recip(den) (DVE), bf16 out.

Hardcoded problem shape: B=4, C=128, H=W=128, hid=128, heads=8, hd=dv=16.
"""

import numpy as np
from contextlib import ExitStack

import concourse.bass as bass
import concourse.tile as tile
from concourse import bacc, mybir
from concourse.bass_utils import run_bass_kernel_spmd

F32 = mybir.dt.float32
BF16 = mybir.dt.bfloat16
F8E5 = mybir.dt.float8e5
AF = mybir.ActivationFunctionType

N_HEADS = 8
HD = 16          # head dim for q/k and v
C = 128          # channels == hid
S = 128 * 128    # pixels per image
SQ = S // 2      # query pixels per core
SK = S           # key pixels per core (KV computed redundantly in the pair —
                 # avoids any cross-core exchange, which both breaks hardware
                 # loops and costs tens of us per AllReduce on this part)
SC = 2048        # super-chunk (pixels)
NKC = SK // SC   # 8
NQC = SQ // SC   # 4

_PROGRAM = None


def _build_program(loop_n=None, unroll_n=None, ident_mod=2, obufs=3, vbufs=3,
                   kv_lag=2):
    nc = bacc.Bacc()
    xs = nc.declare_dram_parameter("xs", [C, SQ], BF16, isOutput=False)
    ys = nc.declare_dram_parameter("ys", [C, SK], BF16, isOutput=False)
    vs = nc.declare_dram_parameter("vs", [128, (SK // 128) * 129], F8E5,
                                   isOutput=False)
    cw = nc.declare_dram_parameter("cw", [128, 4 * 128], BF16, isOutput=False)
    cf = nc.declare_dram_parameter("cf", [128, 139], F32, isOutput=False)
    outp = nc.declare_dram_parameter("outp", [128, SQ], BF16, isOutput=True)

    with tile.TileContext(nc) as tc, ExitStack() as ctx:
        singles = ctx.enter_context(tc.tile_pool(name="singles", bufs=1))
        inpool = ctx.enter_context(tc.tile_pool(name="inpool", bufs=3))
        vpool = ctx.enter_context(tc.tile_pool(name="vpool", bufs=vbufs))
        tpool = ctx.enter_context(tc.tile_pool(name="tpool", bufs=3))
        fpool = ctx.enter_context(tc.tile_pool(name="fpool", bufs=3))
        qpool = ctx.enter_context(tc.tile_pool(name="qpool", bufs=8))
        npool = ctx.enter_context(tc.tile_pool(name="npool", bufs=2))
        opool = ctx.enter_context(tc.tile_pool(name="opool", bufs=obufs))
        ppt = ctx.enter_context(tc.tile_pool(name="ppt", bufs=3, space="PSUM"))
        ppk = ctx.enter_context(tc.tile_pool(name="ppk", bufs=2, space="PSUM"))
        ppkv = ctx.enter_context(tc.tile_pool(name="ppkv", bufs=1, space="PSUM"))

        cwt = singles.tile([128, 4 * 128], BF16)
        nc.sync.dma_start(out=cwt, in_=cw[:])
        wkt_bf = cwt[:, 0:128]
        gl_bf = cwt[:, 128:256]
        bmat_bf = cwt[:, 256:384]
        fmat_bf = cwt[:, 384:512]
        cft = singles.tile([128, 139], F32)
        nc.sync.dma_start(out=cft, in_=cf[:])
        bk_col = cft[:, 0:1]
        gq_col = cft[:, 1:2]
        mask136 = cft[:, 3:139]

        # Tiny ops so each engine observes the consts DMAs once up front
        # (limits per-instruction semaphore-wait slots later).
        presync = ppt.tile([128, 512], F32, tag="tps")
        nc.tensor.matmul(presync[:1, 0:1], lhsT=cwt[:, 0:1], rhs=cwt[:, 0:1],
                         start=True, stop=True)
        dve_sync = singles.tile([128, 1], F32)
        nc.vector.tensor_copy(dve_sync, cft[:, 0:1])
        act_warm = singles.tile([128, 1], F32)
        nc.scalar.activation(act_warm, cft[:, 0:1], AF.Exp)
        pool_sync = singles.tile([128, 1], BF16)
        nc.gpsimd.tensor_copy(pool_sync, cwt[:, 0:1])

        def body():
            # ---- K phase: KV[hm, hv|ksum] accumulated in PSUM ----
            # Software-pipelined over NB blocks of 1024 pixels; engine queues
            # are in-order, so issue order is chosen to keep PE fed:
            # ... proj(j), feats(j-1), kv(j-2) ...
            NB = SK // 1024  # 16 blocks
            HB = 4096        # DMA half-size
            NH = SK // HB    # 4 input-DMA chunks
            kvps = ppkv.tile([128, 129], F32, tag="kv")
            y_ts, v_ts, t_ts, t2_ts, kf_ts = {}, {}, {}, {}, {}

            def k_dma(ih):
                y_t = inpool.tile([128, HB], BF16, tag="inbuf")
                nc.sync.dma_start(out=y_t, in_=ys[:, ih * HB:(ih + 1) * HB])
                v_t = vpool.tile([128, HB // 128, 129], F8E5, tag="v")
                nv = (HB // 128) * 129
                nc.sync.dma_start(out=v_t, in_=vs[:, ih * nv:(ih + 1) * nv])
                y_ts[ih], v_ts[ih] = y_t, v_t

            def k_proj(j):
                ih, jj = divmod(j, HB // 1024)
                t_t = tpool.tile([128, 1024], BF16, tag="t")
                t2_t = tpool.tile([128, 1024], BF16, tag="t2")
                for u in range(2):
                    hs = slice(u * 512, (u + 1) * 512)
                    tps = ppt.tile([128, 512], F32, tag="tps")
                    nc.tensor.matmul(tps, lhsT=wkt_bf,
                                     rhs=y_ts[ih][:, jj * 1024 + u * 512:
                                                  jj * 1024 + (u + 1) * 512],
                                     start=True, stop=True)
                    # balance PSUM->SBUF copies: Act takes 1 in 4, DVE the rest
                    if u == 0 and j % ident_mod == 0:
                        nc.scalar.activation(t_t[:, hs], tps, AF.Identity,
                                             bias=bk_col)
                    else:
                        nc.vector.tensor_scalar_add(t_t[:, hs], tps, bk_col)
                    if u == 0:  # squares: half Pool, half DVE (2x bf16 mode)
                        nc.gpsimd.tensor_mul(t2_t[:, hs], t_t[:, hs], t_t[:, hs])
                    else:
                        nc.vector.tensor_mul(t2_t[:, hs], t_t[:, hs], t_t[:, hs])
                t_ts[j], t2_ts[j] = t_t, t2_t

            def k_feats(j):
                kfps = ppk.tile([128, 1024], F32, tag="kfps")
                for ci in range(8):
                    sl = slice(ci * 128, (ci + 1) * 128)
                    nc.tensor.matmul(kfps[:, sl], lhsT=t_ts[j][:, sl],
                                     rhs=bmat_bf, start=True, stop=False)
                    nc.tensor.matmul(kfps[:, sl], lhsT=t2_ts[j][:, sl],
                                     rhs=fmat_bf, start=False, stop=True)
                kf = fpool.tile([128, 1024], BF16, tag="feat")
                nc.scalar.activation(kf, kfps, AF.Exp)
                kf_ts[j] = kf

            def k_kv(j):
                ih, jj = divmod(j, HB // 1024)
                for ci in range(8):
                    cc = jj * 8 + ci
                    nc.tensor.matmul(kvps, lhsT=kf_ts[j][:, ci * 128:(ci + 1) * 128],
                                     rhs=v_ts[ih][:, cc, :],
                                     start=(j == 0 and ci == 0),
                                     stop=(j == NB - 1 and ci == 7),
                                     skip_group_check=True)

            # Q projection state — q_proj blocks are issued inside the K loop
            # so Act/PE stay dense and the num phase is purely DVE+PE.
            x_ts = []
            qs_ts = {}

            def x_dma(ih):
                x_t = inpool.tile([128, HB], BF16, tag="xin")
                nc.sync.dma_start(out=x_t, in_=xs[:, ih * HB:(ih + 1) * HB])
                x_ts.append(x_t)

            def q_proj(j):
                ih, jj = divmod(j, HB // 1024)
                qs_t = qpool.tile([128, 1024], BF16, tag="qs")
                for u in range(2):
                    qps = ppt.tile([128, 512], F32, tag="tps")
                    nc.tensor.matmul(qps, lhsT=gl_bf,
                                     rhs=x_ts[ih][:, jj * 1024 + u * 512:
                                                  jj * 1024 + (u + 1) * 512],
                                     start=True, stop=True)
                    nc.scalar.activation(qs_t[:, u * 512:(u + 1) * 512], qps,
                                         AF.Exp, bias=gq_col)
                qs_ts[j] = qs_t

            k_dma(0)
            k_proj(0)
            k_proj(1)
            k_feats(0)
            for j in range(2, NB):
                if j % (HB // 1024) == 2 and j // (HB // 1024) + 1 < NH:
                    k_dma(j // (HB // 1024) + 1)
                k_proj(j)
                k_feats(j - 1)
                k_kv(j - 2)
            k_feats(NB - 1)
            k_kv(NB - 2)
            k_kv(NB - 1)
            x_dma(0)
            x_dma(1)

            # ---- kvb straight from the local (full) KV accumulator ----
            kvsh = npool.tile([128, 129], F32, tag="kvsh")
            nc.vector.tensor_copy(kvsh, kvps)

            # ---- kvb: block-diagonal [KV | ksum] bf16 for the num matmul ----
            kvb3 = npool.tile([128, 8, 17], BF16, tag="kvb3")
            m3 = mask136.rearrange("p (h j) -> p h j", h=8)
            nc.vector.tensor_mul(kvb3[:, :, 0:16],
                                 kvsh[:, 0:128].rearrange("p (h j) -> p h j", h=8),
                                 m3[:, :, 0:16])
            nc.vector.tensor_mul(kvb3[:, :, 16:17],
                                 kvsh[:, 128:129, None].to_broadcast([128, 8, 1]),
                                 m3[:, :, 16:17])
            kvb = kvb3.rearrange("p h j -> p (h j)")

            def qs_chunk(cc64):
                return qs_ts[cc64 // 8][:, (cc64 % 8) * 128:(cc64 % 8 + 1) * 128]

            q_proj(0)
            q_proj(1)

            # ---- num/den + divide ----
            nqp = 2  # q_proj blocks issued so far
            for isc in range(NQC):
                out_t = opool.tile([128, SC // 128, 128], BF16, tag="outb")
                cc = 0
                while cc < SC // 128:
                    g = min(7, SC // 128 - cc)
                    need = (isc * 16 + cc + g - 1) // 8 + 1  # one block lookahead
                    while nqp <= min(need, SQ // 1024 - 1):
                        q_proj(nqp)
                        nqp += 1
                    nps = ppk.tile([128, 7, 136], F32, tag="kfps")
                    for i in range(g):
                        nc.tensor.matmul(nps[:, i, :], lhsT=qs_chunk(isc * 16 + cc + i),
                                         rhs=kvb, start=True, stop=True)
                    nps4 = nps[:, 0:g, :].rearrange("p c (h j) -> p c h j", h=8)
                    rden = npool.tile([128, 7, 8], F32, tag="rden")
                    nc.vector.reciprocal(rden[:, 0:g], nps4[:, :, :, 16])
                    nc.vector.tensor_mul(
                        out_t[:, cc:cc + g].rearrange("p c (h j) -> p c h j", h=8),
                        nps4[:, :, :, 0:16],
                        rden[:, 0:g, :, None].to_broadcast([128, g, 8, 16]))
                    cc += g
                nc.gpsimd.dma_start(
                    out=outp[:, isc * SC:(isc + 1) * SC],
                    in_=out_t.rearrange("p k c -> p (k c)"))

        if unroll_n is not None:
            for _ in range(unroll_n):
                body()
        elif loop_n is None:
            body()
        else:
            with tc.For_i(0, loop_n, 1):
                body()

    nc.compile()
    return nc


def _get_program():
    global _PROGRAM
    if _PROGRAM is None:
        _PROGRAM = _build_program()
    return _PROGRAM


def _host_consts(rfs, Wq, bq, Wk, bk):
    import ml_dtypes
    scale = HD ** -0.25  # == 0.5 exactly
    bmat = np.zeros((128, 128), dtype=np.float32)
    fmat = np.zeros((128, 128), dtype=np.float32)
    for h in range(N_HEADS):
        bmat[16 * h:16 * h + 16, 16 * h:16 * h + 16] = rfs[h]
        fmat[16 * h:16 * h + 16, 16 * h:16 * h + 16] = -0.5
    cw = np.zeros((128, 4 * 128), dtype=np.float32)
    cw[:, 0:128] = (scale * Wk).T
    cw[:, 128:256] = (scale * Wq).T @ bmat
    cw[:, 256:384] = bmat
    cw[:, 384:512] = fmat
    cf = np.zeros((128, 139), dtype=np.float32)
    cf[:, 0] = scale * bk
    cf[:, 1] = bmat.T @ (scale * bq)
    for h in range(N_HEADS):
        cf[16 * h:16 * h + 16, 3 + 17 * h:3 + 17 * h + 17] = 1.0
    return cw.astype(ml_dtypes.bfloat16), cf


def make_in_maps(inputs):
    import ml_dtypes
    bf = ml_dtypes.bfloat16
    x = np.asarray(inputs["x"], dtype=np.float32)
    y = np.asarray(inputs["y"], dtype=np.float32)
    cw, cf = _host_consts(np.asarray(inputs["rfs"], dtype=np.float32),
                          np.asarray(inputs["Wq"], dtype=np.float32),
                          np.asarray(inputs["bq"], dtype=np.float32),
                          np.asarray(inputs["Wk"], dtype=np.float32),
                          np.asarray(inputs["bk"], dtype=np.float32))
    B = x.shape[0]
    xr = x.reshape(B, C, S)
    yr = y.reshape(B, C, S)
    in_maps = []
    vs_b, ys_b = {}, {}
    for b in range(B):
        y_pm = yr[b].T                          # [S, C] pixel-major
        v3 = np.concatenate(
            [y_pm.reshape(SK // 128, 128, 128),
             np.ones((SK // 128, 128, 1), np.float32)], axis=2)
        vs_b[b] = np.ascontiguousarray(
            v3.transpose(1, 0, 2).reshape(128, (SK // 128) * 129)).astype(
                ml_dtypes.float8_e5m2)
        ys_b[b] = np.ascontiguousarray(yr[b]).astype(bf)
    for core in range(8):
        b, half = core // 2, core % 2
        s0 = half * SQ
        in_maps.append({
            "xs": np.ascontiguousarray(xr[b][:, s0:s0 + SQ]).astype(bf),
            "ys": ys_b[b],
            "vs": vs_b[b],
            "cw": cw,
            "cf": cf,
        })
    return in_maps


def run(inputs, trace=False, **kwargs):
    in_maps = make_in_maps(inputs)
    nc = _get_program()
    res = run_bass_kernel_spmd(nc, in_maps, list(range(8)), trace=trace, **kwargs)
    B = np.asarray(inputs["x"]).shape[0]
    out = np.empty((B, S, 128), dtype=np.float32)
    for core in range(8):
        b, half = core // 2, core % 2
        s0 = half * SQ
        arr = np.asarray(res.results[core]["outp"], dtype=np.float32)
        out[b, s0:s0 + SQ, :] = (
            arr.reshape(128, NQC, 16, 128).transpose(1, 2, 0, 3).reshape(SQ, 128))
    return out.reshape(np.asarray(inputs["x"]).shape), res


def kernel(**inputs):
    out, _ = run(inputs, trace=False)
    return out
